# revision 39
# baseline (speedup 1.0000x reference)
"""CRF forward kernel for Trainium2, 8 NeuronCores — K=6 rank-1 segments.

The 1024-step recurrence splits into 6 segments (boundaries
0,170,341,512,682,853,1024); products of ~170 random positive matrices
are rank-1 to machine precision (Perron contraction), so middle segments
factor as T ~ f g^T / d from independently seeded forward/backward
chains.  12 chains pack exactly into 3 bundles of 128 partitions
(4 blocks x 32 tags), each advancing one (matmul -> ef-mul) per step:

  A (170 steps): a0 | f3 | g3 | c3   const=g3.END, marker=c3.END
  B (171 steps): f1 | g1 | f2 | g2   no carriers
  C (171 steps): f4 | g4 | c4 | b5   const=g4.END, markers=c4/b5.END

Carrier rows live in the always-zero rows of backward blocks (W's END
column is forbidden).  Injection needs no extra wiring: a marker value
in a backward block's END row propagates through the natural
W[END,:] = exp(trans[END]) row of the scaled stationary (hence the
(L+1)*log lambda repayment).  Const rows self-normalize via a unit
colsum column in the renorm events; dumped factors make every scale
exactly compensable on the host.

Per step: PE runs 3 matmuls and DVE 3 elementwise muls (both ~90% busy;
GPSIMD cannot read PSUM, ACT has no tensor-tensor op).  ef streams are
fp8 (2.7e-4 rel total vs the 2e-2 gate), chunk-DMA'd up front across
both HWDGE rings, first chunk ahead of everything.  Measured: 137.8us
vs the 354.6us meet-in-the-middle baseline.
"""

import os
import sys

import numpy as np
import ml_dtypes

if "/opt/trn_rl_repo" not in sys.path:
    sys.path.insert(0, "/opt/trn_rl_repo")

import concourse.bass as bass
import concourse.tile as tile
from concourse import bacc, mybir
from concourse.bass_utils import run_bass_kernel_spmd

BF = ml_dtypes.bfloat16
F8 = ml_dtypes.float8_e4m3
S, B, T = 1024, 1024, 32
START, END = T - 2, T - 1
NCORES = 8
FD = 128
NK, LAG = 128, 6
BND = [0, 170, 341, 512, 682, 853, 1024]
QA, QB, QC = 170, 171, 171
EV0 = {"A": 68, "B": 89, "C": 110}
NEV = {k: (q - EV0[k] - 1) // NK + 1
       for k, q in (("A", QA), ("B", QB), ("C", QC))}
NCOL = {"A": 5, "B": 4, "C": 5}
BLK = [slice(32 * k, 32 * k + 32) for k in range(4)]
KI = {"A": 0, "B": 1, "C": 2}
# packed small-input layout (col offsets): w | oc | ob | s0 per bundle
PK_W = lambda ki: ki * 128
PK_OC = lambda ki: 384 + ki * 128
PK_OB = lambda ki: 768 + ki * 8
PK_S0 = lambda ki: 792 + ki * 128
PKW = 1176

dt = mybir.dt


def _chunk_bounds(q):
    bounds = [0]
    for inc in (8, 16, 32):
        bounds.append(min(q, bounds[-1] + inc))
    while bounds[-1] < q:
        bounds.append(min(q, bounds[-1] + 128))
    return list(zip(bounds[:-1], bounds[1:]))


def build_program():
    nc = bacc.Bacc("TRN2", target_bir_lowering=False, num_devices=NCORES)

    d = {}
    for k, q in (("A", QA), ("B", QB), ("C", QC)):
        d[f"ef{k}"] = nc.dram_tensor(f"ef{k}", [128, q * FD], dt.float8e4,
                                     kind="ExternalInput")
        d[f"rd{k}"] = nc.dram_tensor(f"rd{k}", [NCOL[k], NEV[k] * FD],
                                     dt.bfloat16, kind="ExternalOutput")
    # all small inputs (w | oc | ob | s0 per bundle) packed into one
    # tensor so the head costs 2 wide DMAs instead of 12 narrow ones
    d["pk"] = nc.dram_tensor("pk", [128, PKW], dt.bfloat16,
                             kind="ExternalInput")
    d["qpk"] = nc.dram_tensor("qpk", [128, 3 * FD], dt.bfloat16,
                              kind="ExternalOutput")

    with tile.TileContext(nc) as tc:
        with (
            tc.tile_pool(name="singles", bufs=1) as singles,
            tc.tile_pool(name="efpool", bufs=1) as efpool,
            tc.tile_pool(name="small", bufs=2) as small,
            tc.tile_pool(name="stA", bufs=3) as stA,
            tc.tile_pool(name="stB", bufs=3) as stB,
            tc.tile_pool(name="stC", bufs=3) as stC,
            tc.tile_pool(name="psA", bufs=2, space="PSUM") as psA,
            tc.tile_pool(name="psB", bufs=2, space="PSUM") as psB,
            tc.tile_pool(name="psC", bufs=2, space="PSUM") as psC,
            tc.tile_pool(name="psE", bufs=1, space="PSUM") as psE,
        ):
            t = {}
            ring = {"A": nc.sync, "B": nc.scalar, "C": nc.sync}
            ring2 = {"A": nc.scalar, "B": nc.sync, "C": nc.scalar}
            # first ef chunk ahead of everything so compute starts early
            spans = {k: _chunk_bounds(q)
                     for k, q in (("A", QA), ("B", QB), ("C", QC))}
            chunks = {}
            efpool_tiles = {}
            for k in ("A", "B", "C"):
                lo, hi = spans[k][0]
                cw = hi - lo
                tl = efpool.tile([128, cw * FD], dt.float8e4,
                                 tag=f"ef{k}0", name=f"ef{k}_0")
                ring[k].dma_start(out=tl[0:64, :],
                                  in_=d[f"ef{k}"].ap()[0:64, lo * FD:hi * FD])
                ring2[k].dma_start(out=tl[64:128, :],
                                   in_=d[f"ef{k}"].ap()[64:128,
                                                        lo * FD:hi * FD])
                chunks[(k, 0)] = (tl, lo, hi)
            pk_t = singles.tile([128, PKW], dt.bfloat16, tag="pk",
                                name="pk_t")
            nc.sync.dma_start(out=pk_t[0:64, :], in_=d["pk"].ap()[0:64, :])
            nc.scalar.dma_start(out=pk_t[64:128, :],
                                in_=d["pk"].ap()[64:128, :])
            for k in ("A", "B", "C"):
                ki = KI[k]
                t[f"w{k}"] = pk_t[:, PK_W(ki):PK_W(ki) + 128]
                t[f"oc{k}"] = pk_t[0:NCOL[k], PK_OC(ki):PK_OC(ki) + 128]
                t[f"ob{k}"] = pk_t[:, PK_OB(ki):PK_OB(ki) + NCOL[k]]
                t[f"rb{k}"] = singles.tile([NCOL[k], NEV[k] * FD],
                                           dt.bfloat16, tag=f"rb{k}",
                                           name=f"rb{k}")

            pools = {"A": stA, "B": stB, "C": stC}
            cur = {}
            for k in ("A", "B", "C"):
                ki = KI[k]
                cur[k] = pk_t[:, PK_S0(ki):PK_S0(ki) + FD]

            nch = len(spans["A"])
            for ch in range(1, nch):
                for k in ("A", "B", "C"):
                    if ch >= len(spans[k]):
                        continue
                    lo, hi = spans[k][ch]
                    cw = hi - lo
                    tl = efpool.tile([128, cw * FD], dt.float8e4,
                                     tag=f"ef{k}{ch}", name=f"ef{k}_{ch}")
                    ring[k].dma_start(
                        out=tl[0:64, :],
                        in_=d[f"ef{k}"].ap()[0:64, lo * FD:hi * FD])
                    ring2[k].dma_start(
                        out=tl[64:128, :],
                        in_=d[f"ef{k}"].ap()[64:128, lo * FD:hi * FD])
                    chunks[(k, ch)] = (tl, lo, hi)

            pend = {"A": {}, "B": {}, "C": {}}
            qpack = singles.tile([128, 3 * FD], dt.bfloat16, tag="qpack",
                                 name="qpack")

            def event(k, i, mul_engine):
                e = (i - EV0[k]) // NK
                ncol = NCOL[k]
                psc = psE.tile([5, FD], dt.float32, tag="psC",
                               name=f"psC{k}_{i}")[0:ncol, :]
                nc.tensor.matmul(psc, t[f"ob{k}"], cur[k], start=True,
                                 stop=True)
                rf = small.tile([5, FD], dt.float32, tag="rf",
                                name=f"rf{k}_{i}")[0:ncol, :]
                nc.vector.reciprocal_approx_fast(out=rf, in_=psc)
                rsb = t[f"rb{k}"][:, e * FD:(e + 1) * FD]
                nc.vector.tensor_copy(rsb, rf)
                if e == NEV[k] - 1:     # dump factors mid-loop, off the tail
                    ring2[k].dma_start(out=d[f"rd{k}"].ap(), in_=t[f"rb{k}"])
                q = {"A": QA, "B": QB, "C": QC}[k]
                if i + LAG < q:
                    pend[k][i + LAG] = rsb

            def step(k, i, ch, lo, tl, ps_pool, mul_engine):
                if i >= EV0[k] and (i - EV0[k]) % NK == 0:
                    event(k, i, mul_engine)
                csl = slice((i - lo) * FD, (i - lo) * FD + FD)
                esl = tl[:, csl]
                if i in pend[k]:
                    rsb = pend[k].pop(i)
                    psr = psE.tile([128, FD], dt.float32, tag="psR",
                                   name=f"psR{k}_{i}")
                    nc.tensor.matmul(psr, t[f"oc{k}"], rsb, start=True,
                                     stop=True)
                    efx = small.tile([128, FD], dt.bfloat16, tag="efx",
                                     name=f"efx{k}_{i}")
                    nc.vector.tensor_mul(efx, psr, esl)
                    esl = efx
                ps = ps_pool.tile([128, FD], dt.float32, tag="ps",
                                  name=f"ps{k}_{i}")
                nc.tensor.matmul(ps, t[f"w{k}"], cur[k], start=True,
                                 stop=True)
                q = {"A": QA, "B": QB, "C": QC}[k]
                if i == q - 1:          # final state -> packed output tile
                    ki = {"A": 0, "B": 1, "C": 2}[k]
                    nxt = qpack[:, ki * FD:(ki + 1) * FD]
                else:
                    nxt = pools[k].tile([128, FD], dt.bfloat16, tag="s",
                                        name=f"s{k}_{i + 1}")
                mul_engine.tensor_mul(nxt, ps, esl)
                cur[k] = nxt

            chi = {"A": 0, "B": 0, "C": 0}
            for i in range(QC):
                for k, q, psp, eng in (("A", QA, psA, nc.vector),
                                       ("B", QB, psB, nc.vector),
                                       ("C", QC, psC, nc.vector)):
                    if i >= q:
                        continue
                    if i >= spans[k][chi[k]][1]:
                        chi[k] += 1
                    tl, lo, hi = chunks[(k, chi[k])]
                    step(k, i, chi[k], lo, tl, psp, eng)

            nc.sync.dma_start(out=d["qpk"].ap(), in_=qpack)

    nc.finalize()
    return nc


def _host_prep(feats, transition, lengths):
    b_tot = feats.shape[1]
    n_cores = b_tot // FD
    b0, b1, b2, b3, b4, b5, b6 = BND
    c_pre = feats.max(axis=2)                                # (S, B)
    Ccum = np.vstack([np.zeros((1, b_tot), np.float64),
                      np.cumsum(c_pre.astype(np.float64), 0)])
    efq = np.exp(feats - c_pre[:, :, None]).astype(np.float32)   # (S,B,T)

    ef_mean = efq.mean(axis=(0, 1)).astype(np.float64)
    Wd = np.exp(transition.astype(np.float64))
    lam = np.abs(np.linalg.eigvals(ef_mean[:, None] * Wd)).max()
    log_lam = float(np.log(lam))
    Ws = Wd / lam
    lhsF = Ws.T
    lhsB = Ws
    eT = np.exp(transition[END].astype(np.float64))

    def bundle_w(kinds, const_blk, mark_blks):
        Wm = np.zeros((128, 128))
        for k, kind in enumerate(kinds):
            Wm[BLK[k], BLK[k]] = lhsF if kind == 'F' else lhsB
        if const_blk is not None:
            ce = 32 * const_blk + END
            Wm[ce, BLK[const_blk]] = 0.0
            Wm[ce, ce] = 1.0
            for mb in mark_blks:
                Wm[ce, 32 * mb + END] = 1.0
        return Wm.astype(BF)

    def bundle_oboc(const_blk, mark_blks, guard_cblks, ncol):
        ob = np.zeros((128, ncol), np.float32)
        oc = np.zeros((ncol, 128), np.float32)
        carrier = []
        if const_blk is not None:
            carrier.append(32 * const_blk + END)
            carrier += [32 * mb + END for mb in mark_blks]
        for k in range(4):
            rows = [r for r in range(32 * k, 32 * k + 32) if r not in carrier]
            ob[rows, k] = 1.0
            oc[k, rows] = 1.0
        if const_blk is not None:
            ce = 32 * const_blk + END
            ob[ce, ncol - 1] = 1.0
            oc[ncol - 1, ce] = 1.0
            for mb in mark_blks:
                oc[ncol - 1, 32 * mb + END] = 1.0
            for cb in guard_cblks:
                ob[ce, cb] = 1.0
        return ob.astype(BF), oc.astype(BF)

    wA = bundle_w(['F', 'F', 'B', 'B'], 2, [3])
    wB = bundle_w(['F', 'B', 'F', 'B'], None, [])
    wC = bundle_w(['F', 'B', 'B', 'B'], 1, [2, 3])
    obA, ocA = bundle_oboc(2, [3], [3], 5)
    obB, ocB = bundle_oboc(None, [], [], 4)
    obC, ocC = bundle_oboc(1, [2, 3], [2, 3], 5)

    Lall = lengths.astype(int)
    in_maps = []
    for core in range(n_cores):
        sl = slice(core * FD, (core + 1) * FD)
        E = np.ascontiguousarray(efq[:, sl, :].transpose(0, 2, 1))  # (S,T,FD)
        Lc = Lall[sl]
        mark = np.zeros((S + 1, FD), np.float32)
        mark[Lc, np.arange(FD)] = 1.0

        def fcols(a, b, q):
            return E[a:b].transpose(1, 0, 2)                 # (T, q, FD)

        def bcols(a, b, q, mark_lo=None, mark_hi=None, zero_end=False):
            ts = b - 2 - np.arange(q)
            out = np.stack([E[tt] if tt >= a else np.ones((T, FD), np.float32)
                            for tt in ts], axis=1)
            if zero_end:
                out[END] = 1.0
            if mark_lo is not None:
                out[END] = np.stack(
                    [mark[tt] if mark_lo <= tt <= mark_hi
                     else np.zeros(FD, np.float32) for tt in ts], axis=0)
            return out

        efA = np.concatenate([
            fcols(b0, b1, QA), fcols(b3, b4, QA),
            bcols(b3, b4, QA, zero_end=True),
            bcols(b3, b4, QA, mark_lo=b3, mark_hi=b4 - 2)], axis=0)
        efB = np.concatenate([
            fcols(b1, b2, QB), bcols(b1, b2, QB),
            fcols(b2, b3, QB), bcols(b2, b3, QB)], axis=0)
        efC = np.concatenate([
            fcols(b4, b5, QC), bcols(b4, b5, QC, zero_end=True),
            bcols(b4, b5, QC, mark_lo=b4, mark_hi=b5 - 2),
            bcols(b5, b6, QC, mark_lo=b5, mark_hi=b6 - 2)], axis=0)

        def bseed(b):
            s = E[b - 1].copy()
            s[END] = 0.0
            return s

        A0 = np.zeros((128, FD), np.float32)
        A0[START] = 1.0
        A0[BLK[1]] = 1.0
        A0[BLK[2]] = bseed(b4)
        A0[64 + END] = 1.0
        A0[96 + END] = mark[b4 - 1]

        B0 = np.zeros((128, FD), np.float32)
        B0[BLK[0]] = 1.0
        B0[BLK[1]] = bseed(b2)
        B0[BLK[2]] = 1.0
        B0[BLK[3]] = bseed(b3)

        C0 = np.zeros((128, FD), np.float32)
        C0[BLK[0]] = 1.0
        C0[BLK[1]] = bseed(b5)
        C0[32 + END] = 1.0
        C0[64 + END] = mark[b5 - 1]
        C0[BLK[3]] = (eT / lam)[:, None].astype(np.float32) \
            * mark[b6][None, :] * E[b6 - 1]
        C0[96 + END] = mark[b6 - 1]

        pk = np.zeros((128, PKW), np.float32)
        for k, w_, ob_, oc_, s0_ in (("A", wA, obA, ocA, A0),
                                     ("B", wB, obB, ocB, B0),
                                     ("C", wC, obC, ocC, C0)):
            ki = KI[k]
            pk[:, PK_W(ki):PK_W(ki) + 128] = w_.astype(np.float32)
            pk[0:NCOL[k], PK_OC(ki):PK_OC(ki) + 128] = oc_.astype(np.float32)
            pk[:, PK_OB(ki):PK_OB(ki) + NCOL[k]] = ob_.astype(np.float32)
            pk[:, PK_S0(ki):PK_S0(ki) + FD] = s0_
        in_maps.append({
            "efA": np.ascontiguousarray(efA).reshape(128, QA * FD).astype(F8),
            "efB": np.ascontiguousarray(efB).reshape(128, QB * FD).astype(F8),
            "efC": np.ascontiguousarray(efC).reshape(128, QC * FD).astype(F8),
            "pk": pk.astype(BF),
        })
    return in_maps, Ccum, log_lam


def _reconstruct(results, Ccum, lengths, log_lam):
    n_cores = len(results)
    b0, b1, b2, b3, b4, b5, b6 = BND
    out = np.zeros(n_cores * FD, np.float64)
    for core in range(n_cores):
        res = results[core]
        qpk = res["qpk"].astype(np.float64)
        Af = qpk[:, 0:FD]
        Bf = qpk[:, FD:2 * FD]
        Cf = qpk[:, 2 * FD:3 * FD]
        lcA = -np.log(np.maximum(res["rdA"].astype(np.float64)
                                 .reshape(5, NEV["A"], FD), 1e-300))
        lcB = -np.log(np.maximum(res["rdB"].astype(np.float64)
                                 .reshape(4, NEV["B"], FD), 1e-300))
        lcC = -np.log(np.maximum(res["rdC"].astype(np.float64)
                                 .reshape(5, NEV["C"], FD), 1e-300))
        bs = core * FD + np.arange(FD)
        L = lengths[bs].astype(int)

        def blk(Xf, k, zero_end=False):
            v = Xf[BLK[k]].copy()
            if zero_end:
                v[END] = 0.0
            return v

        a0 = blk(Af, 0)
        f3 = blk(Af, 1)
        g3 = blk(Af, 2, True)
        c3 = blk(Af, 3, True)
        f1 = blk(Bf, 0)
        g1 = blk(Bf, 1)
        f2 = blk(Bf, 2)
        g2 = blk(Bf, 3)
        f4 = blk(Cf, 0)
        g4 = blk(Cf, 1, True)
        c4 = blk(Cf, 2, True)
        b5v = blk(Cf, 3, True)

        def CC(a, b):
            return Ccum[b, bs] - Ccum[a, bs]

        acc_a0 = CC(b0, b1) + lcA[0].sum(0)
        acc_f3 = CC(b3, b4) + lcA[1].sum(0)
        acc_g3 = CC(b3, b4) + lcA[2].sum(0)
        acc_f1 = CC(b1, b2) + lcB[0].sum(0)
        acc_g1 = CC(b1, b2) + lcB[1].sum(0)
        acc_f2 = CC(b2, b3) + lcB[2].sum(0)
        acc_g2 = CC(b2, b3) + lcB[3].sum(0)
        acc_f4 = CC(b4, b5) + lcC[0].sum(0)
        acc_g4 = CC(b4, b5) + lcC[1].sum(0)

        def acc_c(lc, blk_col, unit_col, a, ev0, n_ev, b, upper):
            i_apps = ev0 + NK * np.arange(n_ev) + LAG
            i_m = (b - 2) - L
            after = (i_apps[:, None] > i_m[None, :])
            inc = np.where(after, lc[blk_col], lc[unit_col])
            return (Ccum[np.minimum(L, upper), bs] - Ccum[a, bs]) + inc.sum(0)

        acc_c3 = acc_c(lcA, 3, 4, b3, EV0["A"], NEV["A"], b4, b4)
        acc_c4 = acc_c(lcC, 2, 4, b4, EV0["C"], NEV["C"], b5, b5)
        acc_b5 = acc_c(lcC, 3, 4, b5, EV0["C"], NEV["C"], b6, b6)

        def logdot(x, ax, y, ay):
            dv = (x * y).sum(0)
            o = np.full(dv.shape, -np.inf)
            nz = dv > 0
            o[nz] = np.log(dv[nz]) + ax[nz] + ay[nz]
            return o

        def lsum(g, acc):
            return np.log(np.maximum(g.sum(0), 1e-300)) + acc

        lg1 = logdot(g1, acc_g1, a0, acc_a0) - lsum(g1, acc_g1)
        lg2 = logdot(g2, acc_g2, f1, acc_f1) - lsum(g2, acc_g2)
        lg3 = logdot(g3, acc_g3, f2, acc_f2) - lsum(g3, acc_g3)
        lg4 = logdot(g4, acc_g4, f3, acc_f3) - lsum(g4, acc_g4)
        t3 = lg1 + lg2 + logdot(c3, acc_c3, f2, acc_f2)
        t4 = lg1 + lg2 + lg3 + logdot(c4, acc_c4, f3, acc_f3)
        t5 = lg1 + lg2 + lg3 + lg4 + logdot(b5v, acc_b5, f4, acc_f4)
        out[bs] = np.logaddexp(np.logaddexp(t3, t4), t5) \
            + (L + 1) * log_lam
    return out


_CACHED_NC = None
LAST_RESULTS = None


def kernel(feats, mask, transition):
    global _CACHED_NC, LAST_RESULTS
    feats = np.asarray(feats, np.float32)
    mask = np.asarray(mask, np.float32)
    transition = np.asarray(transition, np.float32)
    lengths = mask.sum(axis=0).astype(np.int64)

    in_maps, Ccum, log_lam = _host_prep(feats, transition, lengths)
    if _CACHED_NC is None:
        _CACHED_NC = build_program()
    trace = bool(int(os.environ.get("CRF_TRACE", "0")))
    if trace:
        try:
            import types
            from trn_agent_boot.trn_boot import _ntff_profile_via_ctypes
            if "antenv.axon_hooks" not in sys.modules:
                mm_ = types.ModuleType("antenv.axon_hooks")
                mm_._HOOK = None
                mm_.set_axon_ntff_profile_hook = lambda h: setattr(mm_, "_HOOK", h)
                mm_.get_axon_ntff_profile_hook = lambda: mm_._HOOK
                sys.modules["antenv.axon_hooks"] = mm_
            sys.modules["antenv.axon_hooks"].set_axon_ntff_profile_hook(
                _ntff_profile_via_ctypes("/opt/axon/libaxon_pjrt.so"))
        except Exception as e:
            print(f"ntff hook registration failed: {e}")
    res = run_bass_kernel_spmd(_CACHED_NC, in_maps, core_ids=list(range(NCORES)),
                               trace=trace)
    LAST_RESULTS = res
    out = _reconstruct(res.results, Ccum, lengths, log_lam)
    return out.astype(np.float32)


if __name__ == "__main__":
    feats = np.load("/tmp/in_feats.npy")
    mask = np.load("/tmp/in_mask.npy")
    trans = np.load("/tmp/in_transition.npy")
    got = kernel(feats, mask, trans)
    exp = np.load("/tmp/expected.npy")
    rel = np.abs(got - exp) / np.maximum(1.0, np.abs(exp))
    print("max rel:", rel.max(), "mean:", rel.mean())
